# revision 9
# baseline (speedup 1.0000x reference)
"""Trainium2 Bass kernel for nn_ContrastiveLoss (SimCLR + spatial contrastive loss).

Strategy (8-core data parallel):
  - Host: L2-normalize z1/z2/embeddings, build transposed fp8 operand tables
    (x16 pre-scale), gather anchor rows, compute fp64 positive-pair dots.
    Per-core the z table's column halves are rotated by -c*512 so each core's
    self-similarity columns land in the ACT-exp'd column range (exact gram
    cancellation needs ACT-vs-ACT bit equality).
  - Device (per core): fp8 DoubleRow matmuls (K=256 in one instruction, 0.5
    cyc/row) of 1024 simclr rows + 512 spatial rows against 8192-col tables.
    Row-sum-of-exp is split across engines: ACT does exp(x/T)+accum on its
    column share straight out of PSUM; DVE computes exp on the rest via the
    Schraudolph bit-trick (int16(a*x+b) reinterpreted as bf16), the GPSIMD
    (Pool) engine does the first add-tree level, DVE finishes the tree.
    A tiny PE Gram matmul per row-tile reproduces bit-exactly the
    self-similarity terms for the ACT-range corrections.
  - Host: sum_exp = S_raw - corr, log, subtract positives, mean -> [2].

Self-contained: hardcodes shapes from the problem spec.
"""
import sys

for _p in ("/opt/trn_rl_repo", "/root/.axon_site/_ro/trn_rl_repo"):
    if _p not in sys.path:
        sys.path.insert(0, _p)

import numpy as np
import ml_dtypes

import concourse.tile as tile
from concourse import bacc, mybir
from concourse.bass_utils import run_bass_kernel_spmd

TEMPERATURE = 0.07
B = 4096     # simclr batch
D = 256      # projection dim
N = 8192     # num cells (spatial table rows, also 2B simclr table rows)
P = 4096     # num spatial pairs
NCORES = 8
SR = B // NCORES          # 512 simclr pair-rows per core (=> 1024 sim rows)
PR = P // NCORES          # 512 spatial rows per core
RT_SIMCLR = (2 * SR) // 128   # 8 row-tiles
RT_SPATIAL = PR // 128        # 4 row-tiles
RT_TOTAL = RT_SIMCLR + RT_SPATIAL  # 12
NGROUP = 4                # psum groups of 2048 columns
F32 = mybir.dt.float32
BF16 = mybir.dt.bfloat16
I16 = mybir.dt.int16
FP8 = mybir.dt.float8e4
FP8_NP = ml_dtypes.float8_e4m3
FP8_SCALE = 16.0          # operand pre-scale; dots come out x256

# ACT handles simclr local columns [0, A_ACT) and [4096, 4096+A_ACT) per
# row-tile (self-sim columns live there thanks to the per-core rotation);
# DVE's bit-trick exp covers the rest. Spatial rows are all-ACT (their
# self columns are data-dependent, so they must stay on the exact path).
A_ACT = 1792
DVE_W = 2 * (4096 - A_ACT)    # 4608 bit-exp'd columns per simclr row-tile
H1 = DVE_W // 2               # add-tree level sizes
H2, H3, H4, H5 = H1 // 2, H1 // 4, H1 // 8, H1 // 16

# Schraudolph constants for bf16-domain exp via int16(a*x + b):
#   exp(x*INV_T) bits_bf16 ~= 128*(127 - C + x*INV_T*log2(e))
INV_T = 1.0 / (TEMPERATURE * FP8_SCALE * FP8_SCALE)
EXP_A = float(128.0 * np.log2(np.e) * INV_T)
EXP_C = 0.0400            # calibration constant (truncation-aware)
EXP_B = float(128.0 * (127.0 - EXP_C))

_CACHE = {}


def _build_nc():
    nc = bacc.Bacc("TRN2", target_bir_lowering=False)

    zT = nc.dram_tensor("zT", [128, 2, N], FP8, kind="ExternalInput")
    eT = nc.dram_tensor("eT", [128, 2, N], FP8, kind="ExternalInput")
    zTl = nc.dram_tensor("zTl", [128, 2, 2 * SR], FP8, kind="ExternalInput")
    aTl = nc.dram_tensor("aTl", [128, 2, PR], FP8, kind="ExternalInput")
    ident = nc.dram_tensor("ident", [128, 128], F32, kind="ExternalInput")

    sraw_o = nc.dram_tensor("sraw", [128, RT_TOTAL], F32, kind="ExternalOutput")
    corr_o = nc.dram_tensor("corr", [128, RT_TOTAL], F32, kind="ExternalOutput")

    inv_t = float(INV_T)
    DR = mybir.MatmulPerfMode.DoubleRow

    with tile.TileContext(nc) as tc:
        with (
            tc.tile_pool(name="tabs", bufs=1) as tabs,
            tc.tile_pool(name="psum", bufs=2, space="PSUM") as psum,
            tc.tile_pool(name="small", bufs=1) as small,
            tc.tile_pool(name="tmp", bufs=4) as tmpp,
            tc.tile_pool(name="ebuf", bufs=2) as ebufp,
            tc.tile_pool(name="tree", bufs=2) as treep,
        ):
            zTl_t = tabs.tile([128, 2, 2 * SR], FP8)
            aTl_t = tabs.tile([128, 2, PR], FP8)
            ident_t = small.tile([128, 128], F32)
            zT_g = [tabs.tile([128, 2, 2048], FP8, name=f"zTg{g}")
                    for g in range(NGROUP)]
            eT_g = [tabs.tile([128, 2, 2048], FP8, name=f"eTg{g}")
                    for g in range(NGROUP)]
            # Load order = consumption order.
            nc.sync.dma_start(zTl_t[:], zTl[:])
            nc.sync.dma_start(aTl_t[:], aTl[:])
            nc.sync.dma_start(ident_t[:], ident[:])
            for g in range(NGROUP):
                nc.sync.dma_start(zT_g[g][:], zT[:, :, g * 2048:(g + 1) * 2048])
                nc.sync.dma_start(eT_g[g][:], eT[:, :, g * 2048:(g + 1) * 2048])

            sraw_t = small.tile([128, RT_TOTAL], F32)
            corr_t = small.tile([128, RT_TOTAL], F32)

            def lhsT_dr(rt):
                lh, li = (zTl_t, rt) if rt < RT_SIMCLR else (aTl_t, rt - RT_SIMCLR)
                return lh[:, :, li * 128:(li + 1) * 128]

            # All Gram diagonals up front: the diagonal of lhsT.T@lhsT is
            # bitwise-identical to the main matmul's self-similarity element
            # for each row; exp'd identically (on ACT) it cancels exactly.
            pgr = psum.tile([128, 2048], F32, tag="big")
            for grt in range(RT_TOTAL):
                ld = lhsT_dr(grt)
                nc.tensor.matmul(pgr[:, grt * 128:(grt + 1) * 128],
                                 ld, ld, start=True, stop=True, perf_mode=DR)
            gd_all = tmpp.tile([128, RT_TOTAL, 128], F32, tag="gd")
            for grt in range(RT_TOTAL):
                # on DVE: gpsimd has no PSUM port
                nc.vector.tensor_tensor(
                    gd_all[:, grt, :],
                    pgr[:, grt * 128:(grt + 1) * 128],
                    ident_t[:], mybir.AluOpType.mult,
                )

            gdv_all = tmpp.tile([128, RT_TOTAL], F32, tag="gdv")
            nc.vector.tensor_reduce(
                gdv_all[:], gd_all[:],
                axis=mybir.AxisListType.X, op=mybir.AluOpType.add,
            )
            nc.scalar.activation(
                corr_t[:], gdv_all[:],
                mybir.ActivationFunctionType.Exp, scale=inv_t,
            )
            nc.sync.dma_start(corr_o[:], corr_t[:])

            # Persistent per-(row-tile, slot) partial sums. Simclr row-tiles
            # use slots 0 (g0 ACT), 1 (g2 ACT), 2 (DVE tree); spatial use 0-3.
            part_all = small.tile([128, RT_TOTAL, 4], F32)
            nc.vector.memset(part_all[:], 0.0)

            def emit_mains(rt, g):
                """4 DR matmuls for one (row-tile, 2048-col group) -> psum."""
                ld = lhsT_dr(rt)
                tab = zT_g[g] if rt < RT_SIMCLR else eT_g[g]
                pg = psum.tile([128, 2048], F32, tag="big")
                for cc in range(4):
                    nc.tensor.matmul(
                        pg[:, cc * 512:(cc + 1) * 512], ld,
                        tab[:, :, cc * 512:(cc + 1) * 512],
                        start=True, stop=True, perf_mode=DR,
                    )
                return pg

            def act_exp(pg, sl, lo, hi, rt, slot):
                nc.scalar.activation(
                    pg[:, lo:hi], pg[:, lo:hi],
                    mybir.ActivationFunctionType.Exp,
                    scale=inv_t, accum_out=part_all[:, rt, slot:slot + 1],
                )
                _ = sl  # slot bookkeeping handled by caller

            def dve_exp(pg, lo, hi, ebuf, eo):
                w = hi - lo
                nc.vector.tensor_scalar(
                    ebuf[:, eo:eo + w], pg[:, lo:hi],
                    EXP_A, EXP_B,
                    mybir.AluOpType.mult, mybir.AluOpType.add,
                )

            def emit_tree(rt, ebuf):
                """bf16 pairwise add-tree over ebuf [128, DVE_W] -> slot 2."""
                v = ebuf[:].bitcast(BF16)
                l1 = treep.tile([128, H1], BF16, tag="l1")
                nc.gpsimd.tensor_tensor(
                    l1[:], v[:, :H1], v[:, H1:DVE_W], mybir.AluOpType.add)
                l2 = treep.tile([128, H2], BF16, tag="l2")
                nc.vector.tensor_tensor(
                    l2[:], l1[:, :H2], l1[:, H2:H1], mybir.AluOpType.add)
                l3 = treep.tile([128, H3], BF16, tag="l3")
                nc.vector.tensor_tensor(
                    l3[:], l2[:, :H3], l2[:, H3:H2], mybir.AluOpType.add)
                l4 = treep.tile([128, H4], BF16, tag="l4")
                nc.vector.tensor_tensor(
                    l4[:], l3[:, :H4], l3[:, H4:H3], mybir.AluOpType.add)
                l5 = treep.tile([128, H5], BF16, tag="l5")
                nc.vector.tensor_tensor(
                    l5[:], l4[:, :H5], l4[:, H5:H4], mybir.AluOpType.add)
                nc.vector.tensor_reduce(
                    part_all[:, rt, 2:3], l5[:],
                    axis=mybir.AxisListType.X, op=mybir.AluOpType.add,
                )

            # Interleave simclr (ACT+DVE mixed) and spatial (all-ACT) groups
            # so ACT and DVE stay concurrently busy; lag-1 pipeline the trees.
            pend_tree = None
            for rt in range(RT_SIMCLR):
                sp_rt = RT_SIMCLR + rt // 2
                sp_g = (rt % 2) * 2
                ebuf = ebufp.tile([128, DVE_W], I16, tag="ebuf")
                w = 2048 - A_ACT
                # g0: ACT [0,A_ACT), DVE tail
                pg = emit_mains(rt, 0)
                act_exp(pg, None, 0, A_ACT, rt, 0)
                dve_exp(pg, A_ACT, 2048, ebuf, 0)
                # g1: all DVE
                pg = emit_mains(rt, 1)
                dve_exp(pg, 0, 2048, ebuf, w)
                # spatial filler group (all ACT)
                pg = emit_mains(sp_rt, sp_g)
                act_exp(pg, None, 0, 2048, sp_rt, sp_g)
                # g2: ACT [0,A_ACT), DVE tail
                pg = emit_mains(rt, 2)
                act_exp(pg, None, 0, A_ACT, rt, 1)
                dve_exp(pg, A_ACT, 2048, ebuf, w + 2048)
                # g3: all DVE
                pg = emit_mains(rt, 3)
                dve_exp(pg, 0, 2048, ebuf, 2 * w + 2048)
                # spatial filler group (all ACT)
                pg = emit_mains(sp_rt, sp_g + 1)
                act_exp(pg, None, 0, 2048, sp_rt, sp_g + 1)
                if pend_tree is not None:
                    emit_tree(rt - 1, pend_tree)
                pend_tree = ebuf
            emit_tree(RT_SIMCLR - 1, pend_tree)

            nc.vector.tensor_reduce(
                sraw_t[:], part_all[:],
                axis=mybir.AxisListType.X, op=mybir.AluOpType.add,
            )

            nc.sync.dma_start(sraw_o[:], sraw_t[:])

    nc.finalize()
    return nc


def _l2norm(x):
    n = np.maximum(np.linalg.norm(x.astype(np.float32), axis=1, keepdims=True), 1e-12)
    return (x.astype(np.float32) / n).astype(np.float32)


def _pack_T(x):
    """[R, D=256] fp32 -> transposed fp8 operand table [128, 2, R], x16."""
    xT = np.ascontiguousarray(x.T) * np.float32(FP8_SCALE)   # [256, R]
    return np.ascontiguousarray(
        xT.reshape(2, 128, xT.shape[1]).transpose(1, 0, 2)
    ).astype(FP8_NP)


def prepare(z1, z2, embeddings, anchor_idx, neighbor_idx):
    """Host-side prep: returns (in_maps, host_ctx)."""
    z1n = _l2norm(np.asarray(z1))
    z2n = _l2norm(np.asarray(z2))
    en = _l2norm(np.asarray(embeddings))
    ai = np.asarray(anchor_idx).astype(np.int64)
    ni = np.asarray(neighbor_idx).astype(np.int64)

    zcat = np.concatenate([z1n, z2n], axis=0)           # [2B, D]
    zT_p = _pack_T(zcat)                                # [128, 2, 8192] fp8
    eT_p = _pack_T(en)                                  # [128, 2, 8192] fp8
    a_rows = en[ai]                                     # [P, D] fp32
    aT_p = _pack_T(a_rows)                              # [128, 2, 4096] fp8

    # fp64 positive-pair logits (match reference semantics)
    psim = (np.sum(z1n.astype(np.float64) * z2n.astype(np.float64), axis=1)
            / np.float64(np.float32(TEMPERATURE)))      # [B]
    pos = (np.sum(a_rows.astype(np.float64) * en[ni].astype(np.float64), axis=1)
           / np.float64(np.float32(TEMPERATURE)))       # [P]
    eq = (ai == ni).astype(np.float64)                  # [P]

    ident = np.eye(128, dtype=np.float32)
    in_maps = []
    for c in range(NCORES):
        zTl_p = np.ascontiguousarray(np.concatenate(
            [zT_p[:, :, c * SR:(c + 1) * SR],
             zT_p[:, :, B + c * SR:B + (c + 1) * SR]], axis=2))  # [128,2,1024]
        aTl_p = np.ascontiguousarray(aT_p[:, :, c * PR:(c + 1) * PR])  # [128,2,512]
        # rotate each 4096-col half left by c*SR so this core's self-sim
        # columns land at local [0,512) and [4096,4608) -- inside the ACT range
        zT_c = np.ascontiguousarray(np.concatenate([
            np.roll(zT_p[:, :, :B], -c * SR, axis=2),
            np.roll(zT_p[:, :, B:], -c * SR, axis=2)], axis=2))
        in_maps.append({
            "zT": zT_c, "eT": eT_p, "zTl": zTl_p, "aTl": aTl_p, "ident": ident,
        })
    return in_maps, (psim, pos, eq)


def finish(results, host_ctx):
    """Host-side epilogue: assemble the two losses from per-core S_raw/corr."""
    psim, pos, eq = host_ctx
    terms1 = np.empty(2 * B, dtype=np.float64)
    terms2 = np.empty(P, dtype=np.float64)
    for c in range(NCORES):
        S = results[c]["sraw"].astype(np.float64).T.reshape(-1)   # [12*128], idx rt*128+p
        C = results[c]["corr"].astype(np.float64).T.reshape(-1)

        s_sim = S[:2 * SR * 1]  # first 8 tiles = 1024 rows
        c_sim = C[:2 * SR]
        sum_exp = s_sim[:2 * SR] - c_sim[:2 * SR]
        p_loc = psim[c * SR:(c + 1) * SR]
        # local rows [0,512) -> z1 part, [512,1024) -> z2 part; same positives
        terms1[c * SR:(c + 1) * SR] = np.log(sum_exp[:SR]) - p_loc
        terms1[B + c * SR:B + (c + 1) * SR] = np.log(sum_exp[SR:2 * SR]) - p_loc

        s_sp = S[2 * SR:2 * SR + PR]
        c_sp = C[2 * SR:2 * SR + PR]
        g = slice(c * PR, (c + 1) * PR)
        total = s_sp - c_sp + eq[g] * np.exp(pos[g])
        terms2[g] = np.log(total) - pos[g]

    l1 = terms1.mean()
    l2 = terms2.mean()
    return np.array([l1, l2], dtype=np.float32)


def get_nc():
    if "nc" not in _CACHE:
        _CACHE["nc"] = _build_nc()
    return _CACHE["nc"]


def kernel(z1, z2, embeddings, anchor_idx, neighbor_idx):
    in_maps, host_ctx = prepare(z1, z2, embeddings, anchor_idx, neighbor_idx)
    nc = get_nc()
    res = run_bass_kernel_spmd(nc, in_maps, list(range(NCORES)))
    return finish(res.results, host_ctx)


# revision 13
# speedup vs baseline: 1.1665x; 1.1665x over previous
"""Trainium2 Bass kernel for nn_ContrastiveLoss (SimCLR + spatial contrastive loss).

Strategy (8-core data parallel):
  - Host: L2-normalize z1/z2/embeddings, build transposed fp8 operand tables
    (x16 pre-scale), gather anchor rows, compute fp64 positive-pair dots.
    Per-core the z table's column halves are rotated by -c*512 so each core's
    self-similarity columns land in the ACT-exp'd column range (exact gram
    cancellation needs ACT-vs-ACT bit equality).
  - Device (per core): fp8 DoubleRow matmuls (K=256 in one instruction, 0.5
    cyc/row) of 1024 simclr rows + 512 spatial rows against 8192-col tables.
    Row-sum-of-exp is split across engines: ACT does exp(x/T)+accum on its
    column share straight out of PSUM; DVE computes exp on the rest via the
    Schraudolph bit-trick (int16(a*x+b) reinterpreted as bf16), the GPSIMD
    (Pool) engine does the first add-tree level, DVE finishes the tree.
    A tiny PE Gram matmul per row-tile reproduces bit-exactly the
    self-similarity terms for the ACT-range corrections.
  - Host: sum_exp = S_raw - corr, log, subtract positives, mean -> [2].

Self-contained: hardcodes shapes from the problem spec.
"""
import sys

for _p in ("/opt/trn_rl_repo", "/root/.axon_site/_ro/trn_rl_repo"):
    if _p not in sys.path:
        sys.path.insert(0, _p)

import numpy as np
import ml_dtypes

import concourse.tile as tile
from concourse import bacc, mybir
from concourse.bass_utils import run_bass_kernel_spmd

TEMPERATURE = 0.07
B = 4096     # simclr batch
D = 256      # projection dim
N = 8192     # num cells (spatial table rows, also 2B simclr table rows)
P = 4096     # num spatial pairs
NCORES = 8
SR = B // NCORES          # 512 simclr pair-rows per core (=> 1024 sim rows)
PR = P // NCORES          # 512 spatial rows per core
RT_SIMCLR = (2 * SR) // 128   # 8 row-tiles
RT_SPATIAL = PR // 128        # 4 row-tiles
RT_TOTAL = RT_SIMCLR + RT_SPATIAL  # 12
NGROUP = 4                # psum groups of 2048 columns
F32 = mybir.dt.float32
BF16 = mybir.dt.bfloat16
I16 = mybir.dt.int16
FP8 = mybir.dt.float8e4
FP8_NP = ml_dtypes.float8_e4m3
FP8_SCALE = 16.0          # operand pre-scale; dots come out x256

# ACT handles simclr local columns [0, A_ACT) and [4096, 4096+A_ACT) per
# row-tile (self-sim columns live there thanks to the per-core rotation);
# DVE's bit-trick exp covers the rest. Spatial rows are all-ACT (their
# self columns are data-dependent, so they must stay on the exact path).
A_ACT = 1792
DVE_W = 2 * (4096 - A_ACT)    # 4608 bit-exp'd columns per simclr row-tile
H1 = DVE_W // 2               # add-tree level sizes
H2, H3, H4, H5 = H1 // 2, H1 // 4, H1 // 8, H1 // 16

# Schraudolph constants for bf16-domain exp via int16(a*x + b):
#   exp(x*INV_T) bits_bf16 ~= 128*(127 - C + x*INV_T*log2(e))
INV_T = 1.0 / (TEMPERATURE * FP8_SCALE * FP8_SCALE)
EXP_A = float(128.0 * np.log2(np.e) * INV_T)
EXP_C = 0.0400            # calibration constant (truncation-aware)
EXP_B = float(128.0 * (127.0 - EXP_C))

_CACHE = {}


def _build_nc():
    nc = bacc.Bacc("TRN2", target_bir_lowering=False)

    zT = nc.dram_tensor("zT", [128, 2, N], FP8, kind="ExternalInput")
    eT = nc.dram_tensor("eT", [128, 2, N], FP8, kind="ExternalInput")
    zTl = nc.dram_tensor("zTl", [128, 2, 2 * SR], FP8, kind="ExternalInput")
    aTl = nc.dram_tensor("aTl", [128, 2, PR], FP8, kind="ExternalInput")
    ident = nc.dram_tensor("ident", [128, 128], F32, kind="ExternalInput")

    sraw_o = nc.dram_tensor("sraw", [128, RT_TOTAL], F32, kind="ExternalOutput")
    corr_o = nc.dram_tensor("corr", [128, RT_TOTAL], F32, kind="ExternalOutput")

    inv_t = float(INV_T)
    DR = mybir.MatmulPerfMode.DoubleRow

    with tile.TileContext(nc) as tc:
        with (
            tc.tile_pool(name="tabs", bufs=1) as tabs,
            tc.tile_pool(name="psum", bufs=2, space="PSUM") as psum,
            tc.tile_pool(name="small", bufs=1) as small,
            tc.tile_pool(name="tmp", bufs=4) as tmpp,
            tc.tile_pool(name="ebuf", bufs=2) as ebufp,
            tc.tile_pool(name="tree", bufs=2) as treep,
            tc.tile_pool(name="scr", bufs=4) as scrp,
        ):
            zTl_t = tabs.tile([128, 2, 2 * SR], FP8)
            aTl_t = tabs.tile([128, 2, PR], FP8)
            ident_t = small.tile([128, 128], F32)
            zT_g = [tabs.tile([128, 2, 2048], FP8, name=f"zTg{g}")
                    for g in range(NGROUP)]
            eT_g = [tabs.tile([128, 2, 2048], FP8, name=f"eTg{g}")
                    for g in range(NGROUP)]
            # Load order = consumption order.
            nc.sync.dma_start(zTl_t[:], zTl[:])
            nc.sync.dma_start(aTl_t[:], aTl[:])
            nc.sync.dma_start(ident_t[:], ident[:])
            for g in range(NGROUP):
                nc.sync.dma_start(zT_g[g][:], zT[:, :, g * 2048:(g + 1) * 2048])
                nc.sync.dma_start(eT_g[g][:], eT[:, :, g * 2048:(g + 1) * 2048])

            sraw_t = small.tile([128, RT_TOTAL], F32)
            corr_t = small.tile([128, RT_TOTAL], F32)

            def lhsT_dr(rt):
                lh, li = (zTl_t, rt) if rt < RT_SIMCLR else (aTl_t, rt - RT_SIMCLR)
                return lh[:, :, li * 128:(li + 1) * 128]

            # All Gram diagonals up front: the diagonal of lhsT.T@lhsT is
            # bitwise-identical to the main matmul's self-similarity element
            # for each row; exp'd identically (on ACT) it cancels exactly.
            pgr = psum.tile([128, 2048], F32, tag="big")
            for grt in range(RT_TOTAL):
                ld = lhsT_dr(grt)
                nc.tensor.matmul(pgr[:, grt * 128:(grt + 1) * 128],
                                 ld, ld, start=True, stop=True, perf_mode=DR)
            gd_all = tmpp.tile([128, RT_TOTAL, 128], F32, tag="gd")
            for grt in range(RT_TOTAL):
                # on DVE: gpsimd has no PSUM port
                nc.vector.tensor_tensor(
                    gd_all[:, grt, :],
                    pgr[:, grt * 128:(grt + 1) * 128],
                    ident_t[:], mybir.AluOpType.mult,
                )

            gdv_all = tmpp.tile([128, RT_TOTAL], F32, tag="gdv")
            nc.vector.tensor_reduce(
                gdv_all[:], gd_all[:],
                axis=mybir.AxisListType.X, op=mybir.AluOpType.add,
            )
            nc.scalar.activation(
                corr_t[:], gdv_all[:],
                mybir.ActivationFunctionType.Exp, scale=inv_t,
            )
            nc.sync.dma_start(corr_o[:], corr_t[:])

            # Persistent per-(row-tile, slot) partial sums. Simclr row-tiles
            # use slots 0 (g0 ACT), 1 (g2 ACT), 2 (DVE tree); spatial use 0-3.
            part_all = small.tile([128, RT_TOTAL, 4], F32)
            nc.vector.memset(part_all[:], 0.0)

            def emit_mains(rt, g):
                """4 DR matmuls for one (row-tile, 2048-col group) -> psum."""
                ld = lhsT_dr(rt)
                tab = zT_g[g] if rt < RT_SIMCLR else eT_g[g]
                pg = psum.tile([128, 2048], F32, tag="big")
                for cc in range(4):
                    nc.tensor.matmul(
                        pg[:, cc * 512:(cc + 1) * 512], ld,
                        tab[:, :, cc * 512:(cc + 1) * 512],
                        start=True, stop=True, perf_mode=DR,
                    )
                return pg

            def act_exp(pg, lo, hi, rt, slot):
                # out goes to SBUF scratch (dead store): writing exp back
                # into the psum tile would add a cross-engine WAR hazard
                # against DVE's reads of the same tile.
                w = hi - lo
                scr = scrp.tile([128, 2048], BF16, tag="scr")
                nc.scalar.activation(
                    scr[:, :w], pg[:, lo:hi],
                    mybir.ActivationFunctionType.Exp,
                    scale=inv_t, accum_out=part_all[:, rt, slot:slot + 1],
                )

            def dve_exp(pg, lo, hi, ebuf, eo):
                w = hi - lo
                nc.vector.tensor_scalar(
                    ebuf[:, eo:eo + w], pg[:, lo:hi],
                    EXP_A, EXP_B,
                    mybir.AluOpType.mult, mybir.AluOpType.add,
                )

            def emit_tree_pool(ebuf):
                """Pool: first two bf16 add-tree levels over ebuf."""
                v = ebuf[:].bitcast(BF16)
                l1 = treep.tile([128, H1], BF16, tag="l1")
                nc.gpsimd.tensor_tensor(
                    l1[:], v[:, :H1], v[:, H1:DVE_W], mybir.AluOpType.add)
                l2 = treep.tile([128, H2], BF16, tag="l2")
                nc.gpsimd.tensor_tensor(
                    l2[:], l1[:, :H2], l1[:, H2:H1], mybir.AluOpType.add)
                return l2

            def emit_tree_dve(rt, l2):
                """DVE: remaining add-tree levels -> part_all slot 2."""
                l3 = treep.tile([128, H3], BF16, tag="l3")
                nc.vector.tensor_tensor(
                    l3[:], l2[:, :H3], l2[:, H3:H2], mybir.AluOpType.add)
                l4 = treep.tile([128, H4], BF16, tag="l4")
                nc.vector.tensor_tensor(
                    l4[:], l3[:, :H4], l3[:, H4:H3], mybir.AluOpType.add)
                nc.vector.tensor_reduce(
                    part_all[:, rt, 2:3], l4[:],
                    axis=mybir.AxisListType.X, op=mybir.AluOpType.add,
                )

            # Interleave simclr (ACT+DVE mixed) and spatial (all-ACT) groups
            # so ACT and DVE stay concurrently busy; lag-1 pipeline the trees.
            pend_ebuf = None
            for rt in range(RT_SIMCLR):
                sp_rt = RT_SIMCLR + rt // 2
                sp_g = (rt % 2) * 2
                ebuf = ebufp.tile([128, DVE_W], I16, tag="ebuf")
                w = 2048 - A_ACT
                # prior row-tile's tree: Pool levels first (they only need
                # the prior ebuf), DVE finishes after its g1 exp below.
                pend_l2 = (emit_tree_pool(pend_ebuf)
                           if pend_ebuf is not None else None)
                # g0: ACT [0,A_ACT), DVE tail
                pg = emit_mains(rt, 0)
                act_exp(pg, 0, A_ACT, rt, 0)
                dve_exp(pg, A_ACT, 2048, ebuf, 0)
                # g1: all DVE
                pg = emit_mains(rt, 1)
                dve_exp(pg, 0, 2048, ebuf, w)
                # spatial filler group (all ACT)
                pg = emit_mains(sp_rt, sp_g)
                act_exp(pg, 0, 2048, sp_rt, sp_g)
                if pend_l2 is not None:
                    emit_tree_dve(rt - 1, pend_l2)
                # g2: ACT [0,A_ACT), DVE tail
                pg = emit_mains(rt, 2)
                act_exp(pg, 0, A_ACT, rt, 1)
                dve_exp(pg, A_ACT, 2048, ebuf, w + 2048)
                # g3: all DVE
                pg = emit_mains(rt, 3)
                dve_exp(pg, 0, 2048, ebuf, 2 * w + 2048)
                # spatial filler group (all ACT)
                pg = emit_mains(sp_rt, sp_g + 1)
                act_exp(pg, 0, 2048, sp_rt, sp_g + 1)
                pend_ebuf = ebuf
            l2 = emit_tree_pool(pend_ebuf)
            emit_tree_dve(RT_SIMCLR - 1, l2)

            nc.vector.tensor_reduce(
                sraw_t[:], part_all[:],
                axis=mybir.AxisListType.X, op=mybir.AluOpType.add,
            )

            nc.sync.dma_start(sraw_o[:], sraw_t[:])

    nc.finalize()
    return nc


def _l2norm(x):
    n = np.maximum(np.linalg.norm(x.astype(np.float32), axis=1, keepdims=True), 1e-12)
    return (x.astype(np.float32) / n).astype(np.float32)


def _pack_T(x):
    """[R, D=256] fp32 -> transposed fp8 operand table [128, 2, R], x16."""
    xT = np.ascontiguousarray(x.T) * np.float32(FP8_SCALE)   # [256, R]
    return np.ascontiguousarray(
        xT.reshape(2, 128, xT.shape[1]).transpose(1, 0, 2)
    ).astype(FP8_NP)


def prepare(z1, z2, embeddings, anchor_idx, neighbor_idx):
    """Host-side prep: returns (in_maps, host_ctx)."""
    z1n = _l2norm(np.asarray(z1))
    z2n = _l2norm(np.asarray(z2))
    en = _l2norm(np.asarray(embeddings))
    ai = np.asarray(anchor_idx).astype(np.int64)
    ni = np.asarray(neighbor_idx).astype(np.int64)

    zcat = np.concatenate([z1n, z2n], axis=0)           # [2B, D]
    zT_p = _pack_T(zcat)                                # [128, 2, 8192] fp8
    eT_p = _pack_T(en)                                  # [128, 2, 8192] fp8
    a_rows = en[ai]                                     # [P, D] fp32
    aT_p = _pack_T(a_rows)                              # [128, 2, 4096] fp8

    # fp64 positive-pair logits (match reference semantics)
    psim = (np.sum(z1n.astype(np.float64) * z2n.astype(np.float64), axis=1)
            / np.float64(np.float32(TEMPERATURE)))      # [B]
    pos = (np.sum(a_rows.astype(np.float64) * en[ni].astype(np.float64), axis=1)
           / np.float64(np.float32(TEMPERATURE)))       # [P]
    eq = (ai == ni).astype(np.float64)                  # [P]

    ident = np.eye(128, dtype=np.float32)
    in_maps = []
    for c in range(NCORES):
        zTl_p = np.ascontiguousarray(np.concatenate(
            [zT_p[:, :, c * SR:(c + 1) * SR],
             zT_p[:, :, B + c * SR:B + (c + 1) * SR]], axis=2))  # [128,2,1024]
        aTl_p = np.ascontiguousarray(aT_p[:, :, c * PR:(c + 1) * PR])  # [128,2,512]
        # rotate each 4096-col half left by c*SR so this core's self-sim
        # columns land at local [0,512) and [4096,4608) -- inside the ACT range
        zT_c = np.ascontiguousarray(np.concatenate([
            np.roll(zT_p[:, :, :B], -c * SR, axis=2),
            np.roll(zT_p[:, :, B:], -c * SR, axis=2)], axis=2))
        in_maps.append({
            "zT": zT_c, "eT": eT_p, "zTl": zTl_p, "aTl": aTl_p, "ident": ident,
        })
    return in_maps, (psim, pos, eq)


def finish(results, host_ctx):
    """Host-side epilogue: assemble the two losses from per-core S_raw/corr."""
    psim, pos, eq = host_ctx
    terms1 = np.empty(2 * B, dtype=np.float64)
    terms2 = np.empty(P, dtype=np.float64)
    for c in range(NCORES):
        S = results[c]["sraw"].astype(np.float64).T.reshape(-1)   # [12*128], idx rt*128+p
        C = results[c]["corr"].astype(np.float64).T.reshape(-1)

        s_sim = S[:2 * SR * 1]  # first 8 tiles = 1024 rows
        c_sim = C[:2 * SR]
        sum_exp = s_sim[:2 * SR] - c_sim[:2 * SR]
        p_loc = psim[c * SR:(c + 1) * SR]
        # local rows [0,512) -> z1 part, [512,1024) -> z2 part; same positives
        terms1[c * SR:(c + 1) * SR] = np.log(sum_exp[:SR]) - p_loc
        terms1[B + c * SR:B + (c + 1) * SR] = np.log(sum_exp[SR:2 * SR]) - p_loc

        s_sp = S[2 * SR:2 * SR + PR]
        c_sp = C[2 * SR:2 * SR + PR]
        g = slice(c * PR, (c + 1) * PR)
        total = s_sp - c_sp + eq[g] * np.exp(pos[g])
        terms2[g] = np.log(total) - pos[g]

    l1 = terms1.mean()
    l2 = terms2.mean()
    return np.array([l1, l2], dtype=np.float32)


def get_nc():
    if "nc" not in _CACHE:
        _CACHE["nc"] = _build_nc()
    return _CACHE["nc"]


def kernel(z1, z2, embeddings, anchor_idx, neighbor_idx):
    in_maps, host_ctx = prepare(z1, z2, embeddings, anchor_idx, neighbor_idx)
    nc = get_nc()
    res = run_bass_kernel_spmd(nc, in_maps, list(range(NCORES)))
    return finish(res.results, host_ctx)
